# revision 42
# baseline (speedup 1.0000x reference)
"""Causal multi-head attention with RoPE on 8 TRN2 NeuronCores.

Sharding: core c -> (batch b = c//4, head-group g = c%4); each core computes
4 of the 16 heads for one batch element (column-parallel QKV, full causal
attention for its heads, row-parallel O slice); host sums 4 partials.

v2 layout (all matmul operands bf16, psum f32):
  - consolidated DMAs: host packs x/weights into partition-major [128, ...]
    images so each tensor is 1-2 DMA descra (SP queue was the v1 bottleneck).
  - head-PAIR attention: heads (2p, 2p+1) live in partition halves of
    qrot/krot[p]; their score tiles share one 2-bank psum tile [128, 1024]
    so mask + exp are single pair-AP instructions.
  - causal tiles are live-exact (bf16 lifts the f32r N>=256 floor): tile r
    of the diagonal block computes cols [128r:512] with one [128,2,128]
    broadcast-tri mask add.
  - exp (ACT) emits one instruction per (pair, tile); [V|1] @ e gives
    attn and softmax denominator in one accumulating matmul chain.
  - o-proj for chunk j-1 is interleaved into chunk j's first pair loop to
    keep PE dense; output DMA'd as bf16 partials.
"""
import numpy as np

import concourse.bass as bass
from concourse import bacc
import concourse.mybir as mybir
import concourse.tile as tile
from concourse import library_config

F32 = mybir.dt.float32
BF16 = mybir.dt.bfloat16

B, S, D, H, HD = 2, 2048, 1024, 16, 64
NCORES = 8
HPC = 4                # heads per core
CL = HPC * HD          # 256 local channels
THETA = 10000.0
SQ = 512               # s_q chunk width
NJ = S // SQ           # 4 chunks
NKT = S // 128         # 16 s_k tiles
KD = D // 128          # 8 contraction chunks
VW = HD + 1            # 65: V channels + ones column

SWAP_MASK = []
for _i in range(16):
    SWAP_MASK += [2 * _i + 1, 2 * _i]


def _build_body(nc, tc, xP, wqP, wkP, wvP, woP, cosP, sinP, triP, outP):
    Exp = mybir.ActivationFunctionType.Exp
    MUL = mybir.AluOpType.mult
    ADD = mybir.AluOpType.add

    with tc.tile_pool(name="persist", bufs=1) as pp, \
         tc.tile_pool(name="ps_big", bufs=2, space="PSUM") as ps_big, \
         tc.tile_pool(name="ps_pa", bufs=1, space="PSUM") as ps_pa, \
         tc.tile_pool(name="ps_fill", bufs=2, space="PSUM") as ps_fill, \
         tc.tile_pool(name="e_pool", bufs=6) as e_pool, \
         tc.tile_pool(name="rp", bufs=2) as rp, \
         tc.tile_pool(name="div_pool", bufs=2) as div_pool, \
         tc.tile_pool(name="pac_pool", bufs=2) as pac_pool, \
         tc.tile_pool(name="out_pool", bufs=6) as out_pool:
        xt = pp.tile([128, KD * S], BF16, name="xt", tag="xt")
        wq_a = pp.tile([128, KD * CL], BF16, name="wq_a", tag="wq_a")
        wk_a = pp.tile([128, KD * CL], BF16, name="wk_a", tag="wk_a")
        wv_a = pp.tile([128, KD * CL], BF16, name="wv_a", tag="wv_a")
        wo_a = pp.tile([128, 2 * D], BF16, name="wo_a", tag="wo_a")
        cosW = pp.tile([128, S], BF16, name="cosW", tag="cosW")
        sinW = pp.tile([128, S], BF16, name="sinW", tag="sinW")
        tri = pp.tile([128, 128], F32, name="tri", tag="tri")
        qrot = [pp.tile([128, S], BF16, name=f"qrot{i}", tag=f"qrot{i}")
                for i in range(2)]
        krot = [pp.tile([128, S], BF16, name=f"krot{i}", tag=f"krot{i}")
                for i in range(2)]
        v_all = pp.tile([128, NKT * HPC * VW], BF16, name="v_all", tag="v_all")
        # per-(pair, chunk) tiles: a single [128, S] tile per pair creates
        # false subtile deps (late-chunk norm writes serialize against
        # o-proj reads of earlier chunks)
        anorm = [[pp.tile([128, SQ], BF16, name=f"anorm{i}_{j}",
                          tag=f"anorm{i}_{j}") for j in range(NJ)]
                 for i in range(2)]

        nc.gpsimd.load_library(library_config.attn)
        ones_v = v_all[:].rearrange("p (t h w) -> p t h w", t=NKT,
                                    h=HPC)[:, :, :, HD:HD + 1]
        nc.vector.memset(ones_v, 1.0)

        # warmup trickle on zeroed SBUF: keeps the PE ramp-tracker alive
        # through the initial DMA wait (idle > ~3us resets the p-state ramp
        # and the first ~3us of real matmuls would run at 1.5-2x cost).
        # DVE copies chain each matmul to the next so they spread ~230ns
        # apart instead of bursting.
        wrm = pp.tile([128, 8], BF16, name="wrm", tag="wrm")
        wr2 = pp.tile([128, 8], BF16, name="wr2", tag="wr2")
        nc.vector.memset(wrm[:], 0.0)
        pwu = ps_fill.tile([128, 512], F32, name="pwu", tag="fill")
        for _ in range(14):
            nc.tensor.matmul(pwu[0:8, 0:8], wrm[:], wrm[:], start=True,
                             stop=True)
            nc.vector.tensor_copy(wr2[:], wrm[:])
            nc.vector.tensor_copy(wrm[:], wr2[:])

        # ---- input DMAs, ordered to match chunk-0 emission order
        # (Q0, K0, v0-3, Q1, K1) so no PE-queue block exceeds ~3us (a longer
        # stall resets the p-state ramp and the rebuild runs at 1.5-2x cost)
        xv = xt[:].rearrange("p (k s) -> p k s", k=KD)
        xs = xP.rearrange("p (k s) -> p k s", k=KD)
        nc.sync.dma_start(wq_a[:, 0:4 * CL], wqP[:, 0:4 * CL])
        nc.sync.dma_start(xv[:, :, 0:256], xs[:, :, 0:256])
        nc.sync.dma_start(wq_a[:, 4 * CL:], wqP[:, 4 * CL:])
        nc.sync.dma_start(xv[:, :, 256:512], xs[:, :, 256:512])
        nc.sync.dma_start(cosW[:, 0:SQ], cosP[:, 0:SQ])
        nc.sync.dma_start(wk_a[:], wkP)
        nc.sync.dma_start(sinW[:, 0:SQ], sinP[:, 0:SQ])
        nc.sync.dma_start(wv_a[:], wvP)
        nc.sync.dma_start(tri[:], triP)
        nc.sync.dma_start(cosW[:, SQ:], cosP[:, SQ:])
        nc.sync.dma_start(sinW[:, SQ:], sinP[:, SQ:])
        nc.sync.dma_start(xv[:, :, SQ:2 * SQ], xs[:, :, SQ:2 * SQ])
        nc.sync.dma_start(wo_a[:], woP)
        for jn in range(2, NJ):
            nc.sync.dma_start(xv[:, :, SQ * jn:SQ * (jn + 1)],
                              xs[:, :, SQ * jn:SQ * (jn + 1)])

        def proj_unit(jn, w_a, rot, mt):
            # one m-tile of Q or K projection + its RoPE, as a filler thunk
            cs = slice(SQ * jn, SQ * (jn + 1))

            def emit():
                pq = ps_fill.tile([128, 512], F32, name="pq", tag="fill")
                half = pq[:, 0:512]
                splits = 2 if jn == 0 else 1
                w = SQ // splits
                for hb in range(splits):
                    s0 = SQ * jn + hb * w
                    for k in range(KD):
                        nc.tensor.matmul(
                            half[:, hb * w:(hb + 1) * w],
                            w_a[:, k * CL + 128 * mt:k * CL + 128 * (mt + 1)],
                            xt[:, k * S + s0:k * S + s0 + w],
                            start=(k == 0), stop=(k == KD - 1))
                qsw = rp.tile([128, SQ], F32, name="qsw", tag="qsw")
                nc.vector.stream_shuffle(qsw[:], half, SWAP_MASK)
                t1 = rp.tile([128, SQ], BF16, name="t1", tag="t1")
                nc.vector.tensor_tensor(t1[:], half, cosW[:, cs], MUL)
                t2 = rp.tile([128, SQ], BF16, name="t2", tag="t2")
                nc.gpsimd.tensor_tensor(t2[:], qsw[:], sinW[:, cs], MUL)
                nc.vector.tensor_tensor(rot[mt][:, cs], t1[:], t2[:], ADD)
            return emit

        def proj_units(jn):
            # pair-0 tensors first so attention can start sooner
            return [proj_unit(jn, w_a, rot, mt)
                    for mt in range(2)
                    for w_a, rot in ((wq_a, qrot), (wk_a, krot))]

        def v_unit(jn, q4, on_dve=False):
            # V projection for one s_k tile (natural layout), as a thunk
            def emit():
                st = 4 * jn + q4
                pvp = ps_fill.tile([128, 512], F32, name="pvp", tag="fill")
                for k in range(KD):
                    nc.tensor.matmul(
                        pvp[:, 0:256],
                        xt[:, k * S + 128 * st:k * S + 128 * (st + 1)],
                        wv_a[:, k * CL:(k + 1) * CL],
                        start=(k == 0), stop=(k == KD - 1))
                dst = v_all[:].rearrange("p (t h w) -> p t h w", t=NKT,
                                         h=HPC)[:, st:st + 1, :, 0:HD]
                src = pvp[:, 0:256].rearrange("p (t h w) -> p t h w",
                                              t=1, h=HPC)
                if on_dve:
                    # chunk 0: keep the ACT queue clear for the exp stream
                    nc.vector.tensor_copy(dst, src)
                else:
                    nc.scalar.copy(dst, src)
            return emit

        def v_units(jn):
            return [v_unit(jn, q4) for q4 in range(4)]

        def po_unit(jp, u):
            # two o-proj m-tiles (2u, 2u+1) for chunk jp sharing one output
            # DMA (HWDGE descriptor-gen is a serial shared device; halving
            # the DMA count halves that cost)
            def emit():
                ob2 = out_pool.tile([128, 1024], BF16, name="ob2", tag="ob2")
                ov = outP.rearrange("p (m s) -> p m s", m=KD)
                for half in range(2):
                    mt = 2 * u + half
                    if jp == 3 and half == 1:
                        # tail: alternate psum pools (psc pool is free by
                        # then) so the final po chain never waits a slot
                        po = ps_big.tile([128, 1024], F32, name="pob",
                                         tag="big")
                    else:
                        po = ps_fill.tile([128, 512], F32, name="po",
                                          tag="fill")
                    for kt in range(2):
                        nc.tensor.matmul(
                            po[:, 0:512],
                            wo_a[:, kt * D + 128 * mt:kt * D + 128 * (mt + 1)],
                            anorm[kt][jp][:, 0:SQ],
                            start=(kt == 0), stop=(kt == 1))
                    dst = ob2[:, 512 * half:512 * (half + 1)]
                    if jp == 3 and half == 1:
                        nc.scalar.copy(dst, po[:, 0:512])
                    else:
                        nc.vector.tensor_copy(dst, po[:, 0:512])
                    if jp == 3 and u == 3:
                        # final unit: two single-mt DMAs so the very last
                        # transfer is small and starts right after its copy;
                        # first one goes through the idle gpsimd SWDGE queue
                        eng = nc.gpsimd if half == 0 else nc.sync
                        eng.dma_start(
                            ov[:, mt:mt + 1, SQ * jp:SQ * (jp + 1)],
                            dst.rearrange("p (m s) -> p m s", m=1))
                if not (jp == 3 and u == 3):
                    # tail DMAs alternate onto the gpsimd SWDGE queue: the
                    # HWDGE generator is a single serial device (~625ns/DMA)
                    eng = nc.gpsimd if (jp >= 2 and u % 2 == 1) else nc.sync
                    eng.dma_start(
                        ov[:, 2 * u:2 * u + 2, SQ * jp:SQ * (jp + 1)],
                        ob2[:].rearrange("p (m s) -> p m s", m=2))
            return emit

        LOOK = 2
        deficit = [0.0]

        VCOPY_ACT = 612.0

        def pop_fillers(fillers):
            # best-fit: emit the first queued unit that fits the PE deficit.
            # V units insert a 612ns copy into the ACT stream, which extends
            # the attention window by that much -> credit it back.
            while fillers:
                pick = None
                for i, (cost, _, _) in enumerate(fillers):
                    if cost <= deficit[0]:
                        pick = i
                        break
                if pick is None:
                    return
                cost, _, thunk = fillers.pop(pick)
                thunk()
                deficit[0] -= cost
                if cost == 854.0 and thunk.__qualname__.find("v_unit") >= 0:
                    deficit[0] += VCOPY_ACT

        def drain_needed(fillers, level):
            # force-emit every unit that must land before attention chunk
            # `level` (its qrot/krot/v_all inputs are read there)
            rest = []
            for cost, need, thunk in fillers:
                if need <= level:
                    thunk()
                else:
                    rest.append((cost, need, thunk))
            fillers[:] = rest

        def attention_chunk(j, fillers, tail=(), norm_out=None,
                            inline_v=None):
            # fillers: (pe_ns, thunk) work emitted into ACT-gated iterations.
            # tail: units emitted just before the last pair's final PV so
            # they fill the post-last-exp norm window (emitting them after
            # the chunk would queue them behind the norm's sem chain).
            # norm_out: if given, pair-norm chains are appended as thunks
            # instead of emitted (caller runs them after boundary ropes).
            # inline_v: {t: v_unit thunk} emitted just before pair-0's PV(t)
            # (chunk 0 builds V inside the attention stream).
            nt = 4 * (j + 1)
            qs0 = SQ * j
            for p in range(2):
                if p == 1:
                    drain_needed(fillers, j + 0.5)
                pa = ps_pa.tile([128, 1024], F32, name="pa", tag="pa")
                pend = {}

                def qk(t):
                    r = t - 4 * j
                    c0 = 0 if r < 0 else 128 * r
                    psc = ps_big.tile([128, 1024], F32, name="psc", tag="big")
                    for hh in range(2):
                        nc.tensor.matmul(
                            psc[:, 512 * hh + c0:512 * hh + 512],
                            krot[p][64 * hh:64 * (hh + 1),
                                    128 * t:128 * (t + 1)],
                            qrot[p][64 * hh:64 * (hh + 1), qs0 + c0:qs0 + SQ],
                            start=True, stop=True)
                    pend[t] = (psc, c0, r)

                for t in range(min(LOOK, nt)):
                    qk(t)
                for t in range(nt):
                    psc, c0, r = pend.pop(t)
                    if r >= 0:
                        pv = psc[:].rearrange("q (h n) -> q h n",
                                              h=2)[:, :, c0:c0 + 128]
                        trib = tri[:].unsqueeze(1).broadcast_to((128, 2, 128))
                        nc.vector.tensor_tensor(pv, pv, trib, ADD)
                    e = e_pool.tile([128, 1024], BF16, name="e", tag="e")
                    ev = e[:].rearrange("q (h n) -> q h n", h=2)[:, :, c0:SQ]
                    pvv = psc[:].rearrange("q (h n) -> q h n",
                                           h=2)[:, :, c0:SQ]
                    nc.scalar.activation(ev, pvv, Exp, scale=0.125)
                    n_live = 512 - c0
                    pe_ns = 2 * n_live * 0.4167
                    if t + LOOK < nt:
                        rl = t + LOOK - 4 * j
                        pe_ns += 2 * (512 - (0 if rl < 0 else 128 * rl)) \
                            * 0.4167
                    deficit[0] += (2 * n_live * 0.833 + 215) - pe_ns
                    if r >= 0:
                        deficit[0] += 250.0
                    # fillers go BEFORE qk(t+LOOK): the PE queue is in-order
                    # and qk blocks on the psc pool slot (freed by exp(t)),
                    # so anything emitted after it would stall behind it.
                    pop_fillers(fillers)
                    if t + LOOK < nt:
                        qk(t + LOOK)
                    if p == 1 and t == nt - 1:
                        for tu in tail:
                            tu()
                    if inline_v is not None and p == 0 and t in inline_v:
                        inline_v.pop(t)()
                    for hh in range(2):
                        h = 2 * p + hh
                        nc.tensor.matmul(
                            pa[0:VW, 512 * hh + c0:512 * hh + 512],
                            v_all[:, (t * HPC + h) * VW:
                                  (t * HPC + h + 1) * VW],
                            e[:, 512 * hh + c0:512 * hh + 512],
                            start=(t == 0), stop=(t == nt - 1))
                # fast pa release: copy psum -> sbuf, normalize off-psum.
                # Final pair skips the copy: nothing queues behind it.
                last = (j == 3 and p == 1)
                if last:
                    pac = pa
                else:
                    # split evacuation across DVE+ACT so pa frees in ~660ns
                    # (a single DVE copy holds it for ~1.2us)
                    pac = pac_pool.tile([128, 1024], BF16, name="pac",
                                        tag="pac")
                    nc.vector.tensor_copy(pac[:, 0:512], pa[:, 0:512])
                    nc.scalar.copy(pac[:, 512:1024], pa[:, 512:1024])

                def norm_emit(p=p, pac=pac, last=last):
                    # bf16 recip/broadcast: the anorm muls then run with
                    # all-2-byte SBUF operands -> DVE 2x mode (327 vs 594).
                    # 1/Z at bf16 adds ~0.4% noise; tolerance is 2e-2.
                    with nc.allow_low_precision(
                            reason="bf16 softmax denominators, tol 2e-2"):
                        _norm_emit(p, pac, last)

                def _norm_emit(p, pac, last):
                    rcp = div_pool.tile([1, 1024], BF16, name="rcp",
                                        tag="rcp")
                    rb = div_pool.tile([64, 1024], BF16, name="rb", tag="rb")

                    def tickle(src):
                        # tiny matmul reading a norm-chain intermediate:
                        # keeps the PE ramp-tracker alive through the tail
                        # norm window (unlike real fillers, it cannot be
                        # hoisted earlier by the scheduler)
                        tk = ps_big.tile([128, 1024], F32, name="tk",
                                         tag="big")
                        nc.tensor.matmul(tk[0:8, 0:8], src, src, start=True,
                                         stop=True)
                    if last:
                        # tail-critical: split recip so bcast/mul pipeline
                        for hh in range(2):
                            nc.vector.reciprocal(
                                rcp[:, 512 * hh:512 * (hh + 1)],
                                pac[HD:HD + 1, 512 * hh:512 * (hh + 1)])
                    else:
                        nc.vector.reciprocal(rcp[:], pac[HD:HD + 1, :])
                    # split broadcast so mul hh=0 overlaps broadcast hh=1
                    for hh in range(2):
                        nc.gpsimd.partition_broadcast(
                            rb[:, 512 * hh:512 * (hh + 1)],
                            rcp[:, 512 * hh:512 * (hh + 1)])
                        if last:
                            tickle(rb[0:8, 512 * hh:512 * hh + 8])
                        nc.vector.tensor_tensor(
                            anorm[p][j][64 * hh:64 * (hh + 1), 0:SQ],
                            pac[0:HD, 512 * hh:512 * (hh + 1)],
                            rb[:, 512 * hh:512 * (hh + 1)], MUL)
                        if last:
                            tickle(anorm[p][j][0:8, 8 * hh:8 * hh + 8])
                if norm_out is None or j == 3:
                    norm_emit()
                else:
                    # defer the norm chain: its DVE/Pool ops otherwise queue
                    # ahead of the next chunk's boundary-critical rope work
                    norm_out.append(norm_emit)
                # norm window + the pac ACT-half copy extend the window
                deficit[0] += 1800.0
                pop_fillers(fillers)

        # chunk 0 emitted directly; everything else threads through the
        # filler queue so PE stays dense during the ACT-gated attention.
        # Order: pair-0 Q/K first, then V tiles + pair-1 Q/K, so pair-0's
        # rope chain (DVE+gpsimd) overlaps the V/pair-1 matmuls.
        u0 = proj_units(0)
        u0[0]()
        u0[1]()
        for u in v_units(0):
            u()
        u0[2]()
        u0[3]()
        PC, VC, OC2 = 1707.0, 854.0, 854.0
        fillq = []
        fillq += [(PC, 1, u) for u in proj_units(1)]
        fillq += [(VC, 1, u) for u in v_units(1)]
        fillq += [(PC, 2, u) for u in proj_units(2)]
        fillq += [(VC, 2, u) for u in v_units(2)]
        norms = []
        attention_chunk(0, fillq, norm_out=norms)
        drain_needed(fillq, 1)
        for nrm in norms:
            nrm()
        norms = []
        fillq += [(PC, 3, u) for u in proj_units(3)]
        fillq += [(VC, 3, u) for u in v_units(3)]
        fillq += [(OC2, 9, po_unit(0, u)) for u in range(4)]
        attention_chunk(1, fillq, norm_out=norms)
        drain_needed(fillq, 2)
        for nrm in norms:
            nrm()
        norms = []
        fillq += [(OC2, 9, po_unit(1, u)) for u in range(4)]
        attention_chunk(2, fillq, norm_out=norms)
        drain_needed(fillq, 3)
        for nrm in norms:
            nrm()
        # hold back most of po(2): it fills the post-last-exp norm window
        fillq += [(OC2, 9, po_unit(2, 0))]
        attention_chunk(3, fillq,
                        tail=[po_unit(2, u) for u in range(1, 4)])
        for _, _, u in fillq:
            u()
        for u in range(4):
            po_unit(3, u)()


def build_nc():
    nc = bacc.Bacc("TRN2", target_bir_lowering=False, debug=False,
                   num_devices=NCORES)
    xP = nc.dram_tensor("xP", [128, KD * S], BF16, kind="ExternalInput").ap()
    wqP = nc.dram_tensor("wqP", [128, KD * CL], BF16,
                         kind="ExternalInput").ap()
    wkP = nc.dram_tensor("wkP", [128, KD * CL], BF16,
                         kind="ExternalInput").ap()
    wvP = nc.dram_tensor("wvP", [128, KD * CL], BF16,
                         kind="ExternalInput").ap()
    woP = nc.dram_tensor("woP", [128, 2 * D], BF16, kind="ExternalInput").ap()
    cosP = nc.dram_tensor("cosP", [128, S], BF16, kind="ExternalInput").ap()
    sinP = nc.dram_tensor("sinP", [128, S], BF16, kind="ExternalInput").ap()
    triP = nc.dram_tensor("triP", [128, 128], F32, kind="ExternalInput").ap()
    outP = nc.dram_tensor("outP", [128, KD * S], BF16,
                          kind="ExternalOutput").ap()
    with tile.TileContext(nc) as tc:
        _build_body(nc, tc, xP, wqP, wkP, wvP, woP, cosP, sinP, triP, outP)
    nc.compile()
    return nc


def host_constants():
    """RoPE cos/sin tiles (T layout, sign folded into sin) + [128,128] tri."""
    freqs = 1.0 / (THETA ** (np.arange(0, HD, 2, dtype=np.float32)
                             / np.float32(HD)))
    pos = np.arange(S, dtype=np.float32)
    ang = pos[:, None] * freqs[None, :]          # [S, 32]
    cos = np.cos(ang).astype(np.float32)
    sin = np.sin(ang).astype(np.float32)
    rows_i = (np.arange(128) % HD) // 2
    cosT = np.ascontiguousarray(cos[:, rows_i].T)          # [128, S]
    sgn = np.where(np.arange(128) % 2 == 0, -1.0, 1.0).astype(np.float32)
    sinT = np.ascontiguousarray(sin[:, rows_i].T * sgn[:, None])
    p = np.arange(128)[:, None]
    tri = np.where(np.arange(128)[None, :] >= p, 0.0, -1e9).astype(np.float32)
    return cosT, sinT, tri


def _pack(mat, kchunks):
    """[kchunks*128, W] -> [128, kchunks*W] partition-major image."""
    kw = mat.shape[1]
    return np.ascontiguousarray(
        mat.reshape(kchunks, 128, kw).transpose(1, 0, 2).reshape(
            128, kchunks * kw))


def make_in_maps(x, wq, wk, wv, wo):
    import ml_dtypes
    bf = ml_dtypes.bfloat16
    cosT, sinT, tri = host_constants()
    in_maps = []
    for c in range(NCORES):
        b, g = divmod(c, 4)
        cs = slice(CL * g, CL * (g + 1))
        xPm = _pack(np.ascontiguousarray(x[b].T), KD).astype(bf)
        wqPm = _pack(np.ascontiguousarray(wq[cs, :].T), KD).astype(bf)
        wkPm = _pack(np.ascontiguousarray(wk[cs, :].T), KD).astype(bf)
        wvPm = _pack(np.ascontiguousarray(wv[cs, :].T), KD).astype(bf)
        woPm = _pack(np.ascontiguousarray(wo[:, cs].T), 2).astype(bf)
        in_maps.append({
            "xP": xPm, "wqP": wqPm, "wkP": wkPm, "wvP": wvPm, "woP": woPm,
            "cosP": cosT.astype(bf), "sinP": sinT.astype(bf), "triP": tri,
        })
    return in_maps


_CACHE = {}
TRACE = False


def kernel(x, q_proj_weight, k_proj_weight, v_proj_weight, o_proj_weight):
    from concourse.bass_utils import run_bass_kernel_spmd
    x = np.asarray(x, dtype=np.float32)
    in_maps = make_in_maps(x, np.asarray(q_proj_weight, dtype=np.float32),
                           np.asarray(k_proj_weight, dtype=np.float32),
                           np.asarray(v_proj_weight, dtype=np.float32),
                           np.asarray(o_proj_weight, dtype=np.float32))
    if "nc" not in _CACHE:
        _CACHE["nc"] = build_nc()
    res = run_bass_kernel_spmd(_CACHE["nc"], in_maps,
                               core_ids=list(range(NCORES)), trace=TRACE)
    _CACHE["last_results"] = res
    out = np.zeros((B, S, D), dtype=np.float32)
    for c in range(NCORES):
        o = np.asarray(res.results[c]["outP"]).astype(np.float32)
        # o[p, mt*S + s] = partial out[b][s, 128*mt + p]
        o = o.reshape(128, KD, S).transpose(2, 1, 0).reshape(S, D)
        out[c // 4] += o
    return out



# revision 43
# speedup vs baseline: 1.0007x; 1.0007x over previous
"""Causal multi-head attention with RoPE on 8 TRN2 NeuronCores.

Sharding: core c -> (batch b = c//4, head-group g = c%4); each core computes
4 of the 16 heads for one batch element (column-parallel QKV, full causal
attention for its heads, row-parallel O slice); host sums 4 partials.

v2 layout (all matmul operands bf16, psum f32):
  - consolidated DMAs: host packs x/weights into partition-major [128, ...]
    images so each tensor is 1-2 DMA descra (SP queue was the v1 bottleneck).
  - head-PAIR attention: heads (2p, 2p+1) live in partition halves of
    qrot/krot[p]; their score tiles share one 2-bank psum tile [128, 1024]
    so mask + exp are single pair-AP instructions.
  - causal tiles are live-exact (bf16 lifts the f32r N>=256 floor): tile r
    of the diagonal block computes cols [128r:512] with one [128,2,128]
    broadcast-tri mask add.
  - exp (ACT) emits one instruction per (pair, tile); [V|1] @ e gives
    attn and softmax denominator in one accumulating matmul chain.
  - o-proj for chunk j-1 is interleaved into chunk j's first pair loop to
    keep PE dense; output DMA'd as bf16 partials.
"""
import numpy as np

import concourse.bass as bass
from concourse import bacc
import concourse.mybir as mybir
import concourse.tile as tile
from concourse import library_config

F32 = mybir.dt.float32
BF16 = mybir.dt.bfloat16

B, S, D, H, HD = 2, 2048, 1024, 16, 64
NCORES = 8
HPC = 4                # heads per core
CL = HPC * HD          # 256 local channels
THETA = 10000.0
SQ = 512               # s_q chunk width
NJ = S // SQ           # 4 chunks
NKT = S // 128         # 16 s_k tiles
KD = D // 128          # 8 contraction chunks
VW = HD + 1            # 65: V channels + ones column

SWAP_MASK = []
for _i in range(16):
    SWAP_MASK += [2 * _i + 1, 2 * _i]


def _build_body(nc, tc, xP, wqP, wkP, wvP, woP, cosP, sinP, triP, outP):
    Exp = mybir.ActivationFunctionType.Exp
    MUL = mybir.AluOpType.mult
    ADD = mybir.AluOpType.add

    with tc.tile_pool(name="persist", bufs=1) as pp, \
         tc.tile_pool(name="ps_big", bufs=2, space="PSUM") as ps_big, \
         tc.tile_pool(name="ps_pa", bufs=1, space="PSUM") as ps_pa, \
         tc.tile_pool(name="ps_fill", bufs=2, space="PSUM") as ps_fill, \
         tc.tile_pool(name="e_pool", bufs=6) as e_pool, \
         tc.tile_pool(name="rp", bufs=2) as rp, \
         tc.tile_pool(name="div_pool", bufs=2) as div_pool, \
         tc.tile_pool(name="pac_pool", bufs=2) as pac_pool, \
         tc.tile_pool(name="out_pool", bufs=6) as out_pool:
        xt = pp.tile([128, KD * S], BF16, name="xt", tag="xt")
        wq_a = pp.tile([128, KD * CL], BF16, name="wq_a", tag="wq_a")
        wk_a = pp.tile([128, KD * CL], BF16, name="wk_a", tag="wk_a")
        wv_a = pp.tile([128, KD * CL], BF16, name="wv_a", tag="wv_a")
        wo_a = pp.tile([128, 2 * D], BF16, name="wo_a", tag="wo_a")
        cosW = pp.tile([128, S], BF16, name="cosW", tag="cosW")
        sinW = pp.tile([128, S], BF16, name="sinW", tag="sinW")
        tri = pp.tile([128, 128], F32, name="tri", tag="tri")
        qrot = [pp.tile([128, S], BF16, name=f"qrot{i}", tag=f"qrot{i}")
                for i in range(2)]
        krot = [pp.tile([128, S], BF16, name=f"krot{i}", tag=f"krot{i}")
                for i in range(2)]
        v_all = pp.tile([128, NKT * HPC * VW], BF16, name="v_all", tag="v_all")
        # per-(pair, chunk) tiles: a single [128, S] tile per pair creates
        # false subtile deps (late-chunk norm writes serialize against
        # o-proj reads of earlier chunks)
        anorm = [[pp.tile([128, SQ], BF16, name=f"anorm{i}_{j}",
                          tag=f"anorm{i}_{j}") for j in range(NJ)]
                 for i in range(2)]

        nc.gpsimd.load_library(library_config.attn)
        ones_v = v_all[:].rearrange("p (t h w) -> p t h w", t=NKT,
                                    h=HPC)[:, :, :, HD:HD + 1]
        nc.vector.memset(ones_v, 1.0)

        # warmup trickle on zeroed SBUF: keeps the PE ramp-tracker alive
        # through the initial DMA wait (idle > ~3us resets the p-state ramp
        # and the first ~3us of real matmuls would run at 1.5-2x cost).
        # DVE copies chain each matmul to the next so they spread ~230ns
        # apart instead of bursting.
        wrm = pp.tile([128, 8], BF16, name="wrm", tag="wrm")
        wr2 = pp.tile([128, 8], BF16, name="wr2", tag="wr2")
        nc.vector.memset(wrm[:], 0.0)
        pwu = ps_fill.tile([128, 512], F32, name="pwu", tag="fill")
        for _ in range(14):
            nc.tensor.matmul(pwu[0:8, 0:8], wrm[:], wrm[:], start=True,
                             stop=True)
            nc.vector.tensor_copy(wr2[:], wrm[:])
            nc.vector.tensor_copy(wrm[:], wr2[:])

        # ---- input DMAs, ordered to match chunk-0 emission order
        # (Q0, K0, v0-3, Q1, K1) so no PE-queue block exceeds ~3us (a longer
        # stall resets the p-state ramp and the rebuild runs at 1.5-2x cost)
        xv = xt[:].rearrange("p (k s) -> p k s", k=KD)
        xs = xP.rearrange("p (k s) -> p k s", k=KD)
        nc.sync.dma_start(wq_a[:, 0:4 * CL], wqP[:, 0:4 * CL])
        nc.sync.dma_start(xv[:, :, 0:256], xs[:, :, 0:256])
        nc.sync.dma_start(wq_a[:, 4 * CL:], wqP[:, 4 * CL:])
        nc.sync.dma_start(xv[:, :, 256:512], xs[:, :, 256:512])
        nc.sync.dma_start(cosW[:, 0:SQ], cosP[:, 0:SQ])
        nc.sync.dma_start(wk_a[:], wkP)
        nc.sync.dma_start(sinW[:, 0:SQ], sinP[:, 0:SQ])
        nc.sync.dma_start(wv_a[:], wvP)
        nc.sync.dma_start(tri[:], triP)
        nc.sync.dma_start(cosW[:, SQ:], cosP[:, SQ:])
        nc.sync.dma_start(sinW[:, SQ:], sinP[:, SQ:])
        nc.sync.dma_start(xv[:, :, SQ:2 * SQ], xs[:, :, SQ:2 * SQ])
        nc.sync.dma_start(wo_a[:], woP)
        for jn in range(2, NJ):
            nc.sync.dma_start(xv[:, :, SQ * jn:SQ * (jn + 1)],
                              xs[:, :, SQ * jn:SQ * (jn + 1)])

        def proj_unit(jn, w_a, rot, mt):
            # one m-tile of Q or K projection + its RoPE, as a filler thunk
            cs = slice(SQ * jn, SQ * (jn + 1))

            def emit():
                pq = ps_fill.tile([128, 512], F32, name="pq", tag="fill")
                half = pq[:, 0:512]
                splits = 2 if jn == 0 else 1
                w = SQ // splits
                for hb in range(splits):
                    s0 = SQ * jn + hb * w
                    for k in range(KD):
                        nc.tensor.matmul(
                            half[:, hb * w:(hb + 1) * w],
                            w_a[:, k * CL + 128 * mt:k * CL + 128 * (mt + 1)],
                            xt[:, k * S + s0:k * S + s0 + w],
                            start=(k == 0), stop=(k == KD - 1))
                qsw = rp.tile([128, SQ], F32, name="qsw", tag="qsw")
                nc.vector.stream_shuffle(qsw[:], half, SWAP_MASK)
                t1 = rp.tile([128, SQ], BF16, name="t1", tag="t1")
                nc.vector.tensor_tensor(t1[:], half, cosW[:, cs], MUL)
                t2 = rp.tile([128, SQ], BF16, name="t2", tag="t2")
                nc.gpsimd.tensor_tensor(t2[:], qsw[:], sinW[:, cs], MUL)
                nc.vector.tensor_tensor(rot[mt][:, cs], t1[:], t2[:], ADD)
            return emit

        def proj_units(jn):
            # pair-0 tensors first so attention can start sooner
            return [proj_unit(jn, w_a, rot, mt)
                    for mt in range(2)
                    for w_a, rot in ((wq_a, qrot), (wk_a, krot))]

        def v_unit(jn, q4, on_dve=False):
            # V projection for one s_k tile (natural layout), as a thunk
            def emit():
                st = 4 * jn + q4
                pvp = ps_fill.tile([128, 512], F32, name="pvp", tag="fill")
                for k in range(KD):
                    nc.tensor.matmul(
                        pvp[:, 0:256],
                        xt[:, k * S + 128 * st:k * S + 128 * (st + 1)],
                        wv_a[:, k * CL:(k + 1) * CL],
                        start=(k == 0), stop=(k == KD - 1))
                dst = v_all[:].rearrange("p (t h w) -> p t h w", t=NKT,
                                         h=HPC)[:, st:st + 1, :, 0:HD]
                src = pvp[:, 0:256].rearrange("p (t h w) -> p t h w",
                                              t=1, h=HPC)
                if on_dve:
                    # chunk 0: keep the ACT queue clear for the exp stream
                    nc.vector.tensor_copy(dst, src)
                else:
                    nc.scalar.copy(dst, src)
            return emit

        def v_units(jn):
            return [v_unit(jn, q4) for q4 in range(4)]

        def po_unit(jp, u):
            # two o-proj m-tiles (2u, 2u+1) for chunk jp sharing one output
            # DMA (HWDGE descriptor-gen is a serial shared device; halving
            # the DMA count halves that cost)
            def emit():
                ob2 = out_pool.tile([128, 1024], BF16, name="ob2", tag="ob2")
                ov = outP.rearrange("p (m s) -> p m s", m=KD)
                for half in range(2):
                    mt = 2 * u + half
                    if jp == 3 and half == 1:
                        # tail: alternate psum pools (psc pool is free by
                        # then) so the final po chain never waits a slot
                        po = ps_big.tile([128, 1024], F32, name="pob",
                                         tag="big")
                    else:
                        po = ps_fill.tile([128, 512], F32, name="po",
                                          tag="fill")
                    for kt in range(2):
                        nc.tensor.matmul(
                            po[:, 0:512],
                            wo_a[:, kt * D + 128 * mt:kt * D + 128 * (mt + 1)],
                            anorm[kt][jp][:, 0:SQ],
                            start=(kt == 0), stop=(kt == 1))
                    dst = ob2[:, 512 * half:512 * (half + 1)]
                    if jp == 3 and half == 1:
                        nc.scalar.copy(dst, po[:, 0:512])
                    else:
                        nc.vector.tensor_copy(dst, po[:, 0:512])
                    if jp == 3 and u == 3:
                        # final unit: two single-mt DMAs so the very last
                        # transfer is small and starts right after its copy;
                        # first one goes through the idle gpsimd SWDGE queue
                        eng = nc.gpsimd if half == 0 else nc.sync
                        eng.dma_start(
                            ov[:, mt:mt + 1, SQ * jp:SQ * (jp + 1)],
                            dst.rearrange("p (m s) -> p m s", m=1))
                if not (jp == 3 and u == 3):
                    # final-chunk DMAs alternate onto the gpsimd SWDGE queue
                    # (separate generator from the serial HWDGE device); the
                    # po(2) tail keeps SP so Pool.SEQ stays free for the
                    # last-pair norm broadcasts
                    eng = nc.gpsimd if (jp == 3 and u % 2 == 1) else nc.sync
                    eng.dma_start(
                        ov[:, 2 * u:2 * u + 2, SQ * jp:SQ * (jp + 1)],
                        ob2[:].rearrange("p (m s) -> p m s", m=2))
            return emit

        LOOK = 2
        deficit = [0.0]

        VCOPY_ACT = 612.0

        def pop_fillers(fillers):
            # best-fit: emit the first queued unit that fits the PE deficit.
            # V units insert a 612ns copy into the ACT stream, which extends
            # the attention window by that much -> credit it back.
            while fillers:
                pick = None
                for i, (cost, _, _) in enumerate(fillers):
                    if cost <= deficit[0]:
                        pick = i
                        break
                if pick is None:
                    return
                cost, _, thunk = fillers.pop(pick)
                thunk()
                deficit[0] -= cost
                if cost == 854.0 and thunk.__qualname__.find("v_unit") >= 0:
                    deficit[0] += VCOPY_ACT

        def drain_needed(fillers, level):
            # force-emit every unit that must land before attention chunk
            # `level` (its qrot/krot/v_all inputs are read there)
            rest = []
            for cost, need, thunk in fillers:
                if need <= level:
                    thunk()
                else:
                    rest.append((cost, need, thunk))
            fillers[:] = rest

        def attention_chunk(j, fillers, tail=(), norm_out=None,
                            inline_v=None):
            # fillers: (pe_ns, thunk) work emitted into ACT-gated iterations.
            # tail: units emitted just before the last pair's final PV so
            # they fill the post-last-exp norm window (emitting them after
            # the chunk would queue them behind the norm's sem chain).
            # norm_out: if given, pair-norm chains are appended as thunks
            # instead of emitted (caller runs them after boundary ropes).
            # inline_v: {t: v_unit thunk} emitted just before pair-0's PV(t)
            # (chunk 0 builds V inside the attention stream).
            nt = 4 * (j + 1)
            qs0 = SQ * j
            for p in range(2):
                if p == 1:
                    drain_needed(fillers, j + 0.5)
                pa = ps_pa.tile([128, 1024], F32, name="pa", tag="pa")
                pend = {}

                def qk(t):
                    r = t - 4 * j
                    c0 = 0 if r < 0 else 128 * r
                    psc = ps_big.tile([128, 1024], F32, name="psc", tag="big")
                    for hh in range(2):
                        nc.tensor.matmul(
                            psc[:, 512 * hh + c0:512 * hh + 512],
                            krot[p][64 * hh:64 * (hh + 1),
                                    128 * t:128 * (t + 1)],
                            qrot[p][64 * hh:64 * (hh + 1), qs0 + c0:qs0 + SQ],
                            start=True, stop=True)
                    pend[t] = (psc, c0, r)

                for t in range(min(LOOK, nt)):
                    qk(t)
                for t in range(nt):
                    psc, c0, r = pend.pop(t)
                    if r >= 0:
                        pv = psc[:].rearrange("q (h n) -> q h n",
                                              h=2)[:, :, c0:c0 + 128]
                        trib = tri[:].unsqueeze(1).broadcast_to((128, 2, 128))
                        nc.vector.tensor_tensor(pv, pv, trib, ADD)
                    e = e_pool.tile([128, 1024], BF16, name="e", tag="e")
                    ev = e[:].rearrange("q (h n) -> q h n", h=2)[:, :, c0:SQ]
                    pvv = psc[:].rearrange("q (h n) -> q h n",
                                           h=2)[:, :, c0:SQ]
                    nc.scalar.activation(ev, pvv, Exp, scale=0.125)
                    n_live = 512 - c0
                    pe_ns = 2 * n_live * 0.4167
                    if t + LOOK < nt:
                        rl = t + LOOK - 4 * j
                        pe_ns += 2 * (512 - (0 if rl < 0 else 128 * rl)) \
                            * 0.4167
                    deficit[0] += (2 * n_live * 0.833 + 215) - pe_ns
                    if r >= 0:
                        deficit[0] += 250.0
                    # fillers go BEFORE qk(t+LOOK): the PE queue is in-order
                    # and qk blocks on the psc pool slot (freed by exp(t)),
                    # so anything emitted after it would stall behind it.
                    pop_fillers(fillers)
                    if t + LOOK < nt:
                        qk(t + LOOK)
                    if p == 1 and t == nt - 1:
                        for tu in tail:
                            tu()
                    if inline_v is not None and p == 0 and t in inline_v:
                        inline_v.pop(t)()
                    for hh in range(2):
                        h = 2 * p + hh
                        nc.tensor.matmul(
                            pa[0:VW, 512 * hh + c0:512 * hh + 512],
                            v_all[:, (t * HPC + h) * VW:
                                  (t * HPC + h + 1) * VW],
                            e[:, 512 * hh + c0:512 * hh + 512],
                            start=(t == 0), stop=(t == nt - 1))
                # fast pa release: copy psum -> sbuf, normalize off-psum.
                # Final pair skips the copy: nothing queues behind it.
                last = (j == 3 and p == 1)
                if last:
                    pac = pa
                else:
                    # split evacuation across DVE+ACT so pa frees in ~660ns
                    # (a single DVE copy holds it for ~1.2us)
                    pac = pac_pool.tile([128, 1024], BF16, name="pac",
                                        tag="pac")
                    nc.vector.tensor_copy(pac[:, 0:512], pa[:, 0:512])
                    nc.scalar.copy(pac[:, 512:1024], pa[:, 512:1024])

                def norm_emit(p=p, pac=pac, last=last):
                    # bf16 recip/broadcast: the anorm muls then run with
                    # all-2-byte SBUF operands -> DVE 2x mode (327 vs 594).
                    # 1/Z at bf16 adds ~0.4% noise; tolerance is 2e-2.
                    with nc.allow_low_precision(
                            reason="bf16 softmax denominators, tol 2e-2"):
                        _norm_emit(p, pac, last)

                def _norm_emit(p, pac, last):
                    rcp = div_pool.tile([1, 1024], BF16, name="rcp",
                                        tag="rcp")
                    rb = div_pool.tile([64, 1024], BF16, name="rb", tag="rb")

                    def tickle(src):
                        # tiny matmul reading a norm-chain intermediate:
                        # keeps the PE ramp-tracker alive through the tail
                        # norm window (unlike real fillers, it cannot be
                        # hoisted earlier by the scheduler)
                        tk = ps_big.tile([128, 1024], F32, name="tk",
                                         tag="big")
                        nc.tensor.matmul(tk[0:8, 0:8], src, src, start=True,
                                         stop=True)
                    if last:
                        # tail-critical: split recip so bcast/mul pipeline
                        for hh in range(2):
                            nc.vector.reciprocal(
                                rcp[:, 512 * hh:512 * (hh + 1)],
                                pac[HD:HD + 1, 512 * hh:512 * (hh + 1)])
                    else:
                        nc.vector.reciprocal(rcp[:], pac[HD:HD + 1, :])
                    # split broadcast so mul hh=0 overlaps broadcast hh=1
                    for hh in range(2):
                        nc.gpsimd.partition_broadcast(
                            rb[:, 512 * hh:512 * (hh + 1)],
                            rcp[:, 512 * hh:512 * (hh + 1)])
                        if last:
                            tickle(rb[0:8, 512 * hh:512 * hh + 8])
                        nc.vector.tensor_tensor(
                            anorm[p][j][64 * hh:64 * (hh + 1), 0:SQ],
                            pac[0:HD, 512 * hh:512 * (hh + 1)],
                            rb[:, 512 * hh:512 * (hh + 1)], MUL)
                        if last:
                            tickle(anorm[p][j][0:8, 8 * hh:8 * hh + 8])
                if norm_out is None or j == 3:
                    norm_emit()
                else:
                    # defer the norm chain: its DVE/Pool ops otherwise queue
                    # ahead of the next chunk's boundary-critical rope work
                    norm_out.append(norm_emit)
                # norm window + the pac ACT-half copy extend the window
                deficit[0] += 1800.0
                pop_fillers(fillers)

        # chunk 0 emitted directly; everything else threads through the
        # filler queue so PE stays dense during the ACT-gated attention.
        # Order: pair-0 Q/K first, then V tiles + pair-1 Q/K, so pair-0's
        # rope chain (DVE+gpsimd) overlaps the V/pair-1 matmuls.
        u0 = proj_units(0)
        u0[0]()
        u0[1]()
        for u in v_units(0):
            u()
        u0[2]()
        u0[3]()
        PC, VC, OC2 = 1707.0, 854.0, 854.0
        fillq = []
        fillq += [(PC, 1, u) for u in proj_units(1)]
        fillq += [(VC, 1, u) for u in v_units(1)]
        fillq += [(PC, 2, u) for u in proj_units(2)]
        fillq += [(VC, 2, u) for u in v_units(2)]
        norms = []
        attention_chunk(0, fillq, norm_out=norms)
        drain_needed(fillq, 1)
        for nrm in norms:
            nrm()
        norms = []
        fillq += [(PC, 3, u) for u in proj_units(3)]
        fillq += [(VC, 3, u) for u in v_units(3)]
        fillq += [(OC2, 9, po_unit(0, u)) for u in range(4)]
        attention_chunk(1, fillq, norm_out=norms)
        drain_needed(fillq, 2)
        for nrm in norms:
            nrm()
        norms = []
        fillq += [(OC2, 9, po_unit(1, u)) for u in range(4)]
        attention_chunk(2, fillq, norm_out=norms)
        drain_needed(fillq, 3)
        for nrm in norms:
            nrm()
        # hold back most of po(2): it fills the post-last-exp norm window
        fillq += [(OC2, 9, po_unit(2, 0))]
        attention_chunk(3, fillq,
                        tail=[po_unit(2, u) for u in range(1, 4)])
        for _, _, u in fillq:
            u()
        for u in range(4):
            po_unit(3, u)()


def build_nc():
    nc = bacc.Bacc("TRN2", target_bir_lowering=False, debug=False,
                   num_devices=NCORES)
    xP = nc.dram_tensor("xP", [128, KD * S], BF16, kind="ExternalInput").ap()
    wqP = nc.dram_tensor("wqP", [128, KD * CL], BF16,
                         kind="ExternalInput").ap()
    wkP = nc.dram_tensor("wkP", [128, KD * CL], BF16,
                         kind="ExternalInput").ap()
    wvP = nc.dram_tensor("wvP", [128, KD * CL], BF16,
                         kind="ExternalInput").ap()
    woP = nc.dram_tensor("woP", [128, 2 * D], BF16, kind="ExternalInput").ap()
    cosP = nc.dram_tensor("cosP", [128, S], BF16, kind="ExternalInput").ap()
    sinP = nc.dram_tensor("sinP", [128, S], BF16, kind="ExternalInput").ap()
    triP = nc.dram_tensor("triP", [128, 128], F32, kind="ExternalInput").ap()
    outP = nc.dram_tensor("outP", [128, KD * S], BF16,
                          kind="ExternalOutput").ap()
    with tile.TileContext(nc) as tc:
        _build_body(nc, tc, xP, wqP, wkP, wvP, woP, cosP, sinP, triP, outP)
    nc.compile()
    return nc


def host_constants():
    """RoPE cos/sin tiles (T layout, sign folded into sin) + [128,128] tri."""
    freqs = 1.0 / (THETA ** (np.arange(0, HD, 2, dtype=np.float32)
                             / np.float32(HD)))
    pos = np.arange(S, dtype=np.float32)
    ang = pos[:, None] * freqs[None, :]          # [S, 32]
    cos = np.cos(ang).astype(np.float32)
    sin = np.sin(ang).astype(np.float32)
    rows_i = (np.arange(128) % HD) // 2
    cosT = np.ascontiguousarray(cos[:, rows_i].T)          # [128, S]
    sgn = np.where(np.arange(128) % 2 == 0, -1.0, 1.0).astype(np.float32)
    sinT = np.ascontiguousarray(sin[:, rows_i].T * sgn[:, None])
    p = np.arange(128)[:, None]
    tri = np.where(np.arange(128)[None, :] >= p, 0.0, -1e9).astype(np.float32)
    return cosT, sinT, tri


def _pack(mat, kchunks):
    """[kchunks*128, W] -> [128, kchunks*W] partition-major image."""
    kw = mat.shape[1]
    return np.ascontiguousarray(
        mat.reshape(kchunks, 128, kw).transpose(1, 0, 2).reshape(
            128, kchunks * kw))


def make_in_maps(x, wq, wk, wv, wo):
    import ml_dtypes
    bf = ml_dtypes.bfloat16
    cosT, sinT, tri = host_constants()
    in_maps = []
    for c in range(NCORES):
        b, g = divmod(c, 4)
        cs = slice(CL * g, CL * (g + 1))
        xPm = _pack(np.ascontiguousarray(x[b].T), KD).astype(bf)
        wqPm = _pack(np.ascontiguousarray(wq[cs, :].T), KD).astype(bf)
        wkPm = _pack(np.ascontiguousarray(wk[cs, :].T), KD).astype(bf)
        wvPm = _pack(np.ascontiguousarray(wv[cs, :].T), KD).astype(bf)
        woPm = _pack(np.ascontiguousarray(wo[:, cs].T), 2).astype(bf)
        in_maps.append({
            "xP": xPm, "wqP": wqPm, "wkP": wkPm, "wvP": wvPm, "woP": woPm,
            "cosP": cosT.astype(bf), "sinP": sinT.astype(bf), "triP": tri,
        })
    return in_maps


_CACHE = {}
TRACE = False


def kernel(x, q_proj_weight, k_proj_weight, v_proj_weight, o_proj_weight):
    from concourse.bass_utils import run_bass_kernel_spmd
    x = np.asarray(x, dtype=np.float32)
    in_maps = make_in_maps(x, np.asarray(q_proj_weight, dtype=np.float32),
                           np.asarray(k_proj_weight, dtype=np.float32),
                           np.asarray(v_proj_weight, dtype=np.float32),
                           np.asarray(o_proj_weight, dtype=np.float32))
    if "nc" not in _CACHE:
        _CACHE["nc"] = build_nc()
    res = run_bass_kernel_spmd(_CACHE["nc"], in_maps,
                               core_ids=list(range(NCORES)), trace=TRACE)
    _CACHE["last_results"] = res
    out = np.zeros((B, S, D), dtype=np.float32)
    for c in range(NCORES):
        o = np.asarray(res.results[c]["outP"]).astype(np.float32)
        # o[p, mt*S + s] = partial out[b][s, 128*mt + p]
        o = o.reshape(128, KD, S).transpose(2, 1, 0).reshape(S, D)
        out[c // 4] += o
    return out



# revision 46
# speedup vs baseline: 1.0196x; 1.0189x over previous
"""Causal multi-head attention with RoPE on 8 TRN2 NeuronCores.

Sharding: core c -> (batch b = c//4, head-group g = c%4); each core computes
4 of the 16 heads for one batch element (column-parallel QKV, full causal
attention for its heads, row-parallel O slice); host sums 4 partials.

v2 layout (all matmul operands bf16, psum f32):
  - consolidated DMAs: host packs x/weights into partition-major [128, ...]
    images so each tensor is 1-2 DMA descra (SP queue was the v1 bottleneck).
  - head-PAIR attention: heads (2p, 2p+1) live in partition halves of
    qrot/krot[p]; their score tiles share one 2-bank psum tile [128, 1024]
    so mask + exp are single pair-AP instructions.
  - causal tiles are live-exact (bf16 lifts the f32r N>=256 floor): tile r
    of the diagonal block computes cols [128r:512] with one [128,2,128]
    broadcast-tri mask add.
  - exp (ACT) emits one instruction per (pair, tile); [V|1] @ e gives
    attn and softmax denominator in one accumulating matmul chain.
  - o-proj for chunk j-1 is interleaved into chunk j's first pair loop to
    keep PE dense; output DMA'd as bf16 partials.
"""
import numpy as np

import concourse.bass as bass
from concourse import bacc
import concourse.mybir as mybir
import concourse.tile as tile
from concourse import library_config

F32 = mybir.dt.float32
BF16 = mybir.dt.bfloat16

B, S, D, H, HD = 2, 2048, 1024, 16, 64
NCORES = 8
HPC = 4                # heads per core
CL = HPC * HD          # 256 local channels
THETA = 10000.0
SQ = 512               # s_q chunk width
NJ = S // SQ           # 4 chunks
NKT = S // 128         # 16 s_k tiles
KD = D // 128          # 8 contraction chunks
VW = HD + 1            # 65: V channels + ones column

SWAP_MASK = []
for _i in range(16):
    SWAP_MASK += [2 * _i + 1, 2 * _i]

import os as _os
NORM_CREDIT = float(_os.environ.get("K_NORM_CREDIT", "1800"))
VCOPY_ACT = float(_os.environ.get("K_VCOPY_ACT", "612"))
LOOKP = int(_os.environ.get("K_LOOK", "3"))


def _build_body(nc, tc, xP, wqP, wkP, wvP, woP, cosP, sinP, triP, outP):
    Exp = mybir.ActivationFunctionType.Exp
    MUL = mybir.AluOpType.mult
    ADD = mybir.AluOpType.add

    with tc.tile_pool(name="persist", bufs=1) as pp, \
         tc.tile_pool(name="ps_big", bufs=2, space="PSUM") as ps_big, \
         tc.tile_pool(name="ps_pa", bufs=1, space="PSUM") as ps_pa, \
         tc.tile_pool(name="ps_fill", bufs=2, space="PSUM") as ps_fill, \
         tc.tile_pool(name="e_pool", bufs=6) as e_pool, \
         tc.tile_pool(name="rp", bufs=2) as rp, \
         tc.tile_pool(name="div_pool", bufs=2) as div_pool, \
         tc.tile_pool(name="pac_pool", bufs=2) as pac_pool, \
         tc.tile_pool(name="out_pool", bufs=6) as out_pool:
        xt = pp.tile([128, KD * S], BF16, name="xt", tag="xt")
        wq_a = pp.tile([128, KD * CL], BF16, name="wq_a", tag="wq_a")
        wk_a = pp.tile([128, KD * CL], BF16, name="wk_a", tag="wk_a")
        wv_a = pp.tile([128, KD * CL], BF16, name="wv_a", tag="wv_a")
        wo_a = pp.tile([128, 2 * D], BF16, name="wo_a", tag="wo_a")
        cosW = pp.tile([128, S], BF16, name="cosW", tag="cosW")
        sinW = pp.tile([128, S], BF16, name="sinW", tag="sinW")
        tri = pp.tile([128, 128], F32, name="tri", tag="tri")
        qrot = [pp.tile([128, S], BF16, name=f"qrot{i}", tag=f"qrot{i}")
                for i in range(2)]
        krot = [pp.tile([128, S], BF16, name=f"krot{i}", tag=f"krot{i}")
                for i in range(2)]
        v_all = pp.tile([128, NKT * HPC * VW], BF16, name="v_all", tag="v_all")
        # per-(pair, chunk) tiles: a single [128, S] tile per pair creates
        # false subtile deps (late-chunk norm writes serialize against
        # o-proj reads of earlier chunks)
        anorm = [[pp.tile([128, SQ], BF16, name=f"anorm{i}_{j}",
                          tag=f"anorm{i}_{j}") for j in range(NJ)]
                 for i in range(2)]

        nc.gpsimd.load_library(library_config.attn)
        ones_v = v_all[:].rearrange("p (t h w) -> p t h w", t=NKT,
                                    h=HPC)[:, :, :, HD:HD + 1]
        nc.vector.memset(ones_v, 1.0)

        # warmup trickle on zeroed SBUF: keeps the PE ramp-tracker alive
        # through the initial DMA wait (idle > ~3us resets the p-state ramp
        # and the first ~3us of real matmuls would run at 1.5-2x cost).
        # DVE copies chain each matmul to the next so they spread ~230ns
        # apart instead of bursting.
        wrm = pp.tile([128, 8], BF16, name="wrm", tag="wrm")
        wr2 = pp.tile([128, 8], BF16, name="wr2", tag="wr2")
        nc.vector.memset(wrm[:], 0.0)
        pwu = ps_fill.tile([128, 512], F32, name="pwu", tag="fill")
        for _ in range(14):
            nc.tensor.matmul(pwu[0:8, 0:8], wrm[:], wrm[:], start=True,
                             stop=True)
            nc.vector.tensor_copy(wr2[:], wrm[:])
            nc.vector.tensor_copy(wrm[:], wr2[:])

        # ---- input DMAs, ordered to match chunk-0 emission order
        # (Q0, K0, v0-3, Q1, K1) so no PE-queue block exceeds ~3us (a longer
        # stall resets the p-state ramp and the rebuild runs at 1.5-2x cost)
        xv = xt[:].rearrange("p (k s) -> p k s", k=KD)
        xs = xP.rearrange("p (k s) -> p k s", k=KD)
        nc.sync.dma_start(wq_a[:, 0:4 * CL], wqP[:, 0:4 * CL])
        nc.sync.dma_start(xv[:, :, 0:256], xs[:, :, 0:256])
        nc.sync.dma_start(wq_a[:, 4 * CL:], wqP[:, 4 * CL:])
        nc.sync.dma_start(xv[:, :, 256:512], xs[:, :, 256:512])
        nc.sync.dma_start(cosW[:, 0:SQ], cosP[:, 0:SQ])
        nc.sync.dma_start(wk_a[:], wkP)
        nc.sync.dma_start(sinW[:, 0:SQ], sinP[:, 0:SQ])
        nc.sync.dma_start(wv_a[:], wvP)
        nc.sync.dma_start(tri[:], triP)
        nc.sync.dma_start(cosW[:, SQ:], cosP[:, SQ:])
        nc.sync.dma_start(sinW[:, SQ:], sinP[:, SQ:])
        nc.sync.dma_start(xv[:, :, SQ:2 * SQ], xs[:, :, SQ:2 * SQ])
        nc.sync.dma_start(wo_a[:], woP)
        for jn in range(2, NJ):
            nc.sync.dma_start(xv[:, :, SQ * jn:SQ * (jn + 1)],
                              xs[:, :, SQ * jn:SQ * (jn + 1)])

        def proj_unit(jn, w_a, rot, mt):
            # one m-tile of Q or K projection + its RoPE, as a filler thunk
            cs = slice(SQ * jn, SQ * (jn + 1))

            def emit():
                pq = ps_fill.tile([128, 512], F32, name="pq", tag="fill")
                half = pq[:, 0:512]
                splits = 2 if jn == 0 else 1
                w = SQ // splits
                for hb in range(splits):
                    s0 = SQ * jn + hb * w
                    for k in range(KD):
                        nc.tensor.matmul(
                            half[:, hb * w:(hb + 1) * w],
                            w_a[:, k * CL + 128 * mt:k * CL + 128 * (mt + 1)],
                            xt[:, k * S + s0:k * S + s0 + w],
                            start=(k == 0), stop=(k == KD - 1))
                qsw = rp.tile([128, SQ], F32, name="qsw", tag="qsw")
                nc.vector.stream_shuffle(qsw[:], half, SWAP_MASK)
                t1 = rp.tile([128, SQ], BF16, name="t1", tag="t1")
                nc.vector.tensor_tensor(t1[:], half, cosW[:, cs], MUL)
                t2 = rp.tile([128, SQ], BF16, name="t2", tag="t2")
                nc.gpsimd.tensor_tensor(t2[:], qsw[:], sinW[:, cs], MUL)
                nc.vector.tensor_tensor(rot[mt][:, cs], t1[:], t2[:], ADD)
            return emit

        def proj_units(jn):
            # pair-0 tensors first so attention can start sooner
            return [proj_unit(jn, w_a, rot, mt)
                    for mt in range(2)
                    for w_a, rot in ((wq_a, qrot), (wk_a, krot))]

        def v_unit(jn, q4, on_dve=False):
            # V projection for one s_k tile (natural layout), as a thunk
            def emit():
                st = 4 * jn + q4
                pvp = ps_fill.tile([128, 512], F32, name="pvp", tag="fill")
                for k in range(KD):
                    nc.tensor.matmul(
                        pvp[:, 0:256],
                        xt[:, k * S + 128 * st:k * S + 128 * (st + 1)],
                        wv_a[:, k * CL:(k + 1) * CL],
                        start=(k == 0), stop=(k == KD - 1))
                dst = v_all[:].rearrange("p (t h w) -> p t h w", t=NKT,
                                         h=HPC)[:, st:st + 1, :, 0:HD]
                src = pvp[:, 0:256].rearrange("p (t h w) -> p t h w",
                                              t=1, h=HPC)
                if on_dve:
                    # chunk 0: keep the ACT queue clear for the exp stream
                    nc.vector.tensor_copy(dst, src)
                else:
                    nc.scalar.copy(dst, src)
            return emit

        def v_units(jn):
            return [v_unit(jn, q4) for q4 in range(4)]

        def po_unit(jp, u):
            # two o-proj m-tiles (2u, 2u+1) for chunk jp sharing one output
            # DMA (HWDGE descriptor-gen is a serial shared device; halving
            # the DMA count halves that cost)
            def emit():
                ob2 = out_pool.tile([128, 1024], BF16, name="ob2", tag="ob2")
                ov = outP.rearrange("p (m s) -> p m s", m=KD)
                for half in range(2):
                    mt = 2 * u + half
                    if jp == 3 and half == 1:
                        # tail: alternate psum pools (psc pool is free by
                        # then) so the final po chain never waits a slot
                        po = ps_big.tile([128, 1024], F32, name="pob",
                                         tag="big")
                    else:
                        po = ps_fill.tile([128, 512], F32, name="po",
                                          tag="fill")
                    for kt in range(2):
                        nc.tensor.matmul(
                            po[:, 0:512],
                            wo_a[:, kt * D + 128 * mt:kt * D + 128 * (mt + 1)],
                            anorm[kt][jp][:, 0:SQ],
                            start=(kt == 0), stop=(kt == 1))
                    dst = ob2[:, 512 * half:512 * (half + 1)]
                    if jp == 3 and half == 1:
                        nc.scalar.copy(dst, po[:, 0:512])
                    else:
                        nc.vector.tensor_copy(dst, po[:, 0:512])
                    if jp == 3 and u == 3:
                        # final unit: two single-mt DMAs so the very last
                        # transfer is small and starts right after its copy;
                        # first one goes through the idle gpsimd SWDGE queue
                        eng = nc.gpsimd if half == 0 else nc.sync
                        eng.dma_start(
                            ov[:, mt:mt + 1, SQ * jp:SQ * (jp + 1)],
                            dst.rearrange("p (m s) -> p m s", m=1))
                if not (jp == 3 and u == 3):
                    # final-chunk DMAs alternate onto the gpsimd SWDGE queue
                    # (separate generator from the serial HWDGE device); the
                    # po(2) tail keeps SP so Pool.SEQ stays free for the
                    # last-pair norm broadcasts
                    eng = nc.gpsimd if (jp == 3 and u % 2 == 1) else nc.sync
                    eng.dma_start(
                        ov[:, 2 * u:2 * u + 2, SQ * jp:SQ * (jp + 1)],
                        ob2[:].rearrange("p (m s) -> p m s", m=2))
            return emit

        LOOK = LOOKP
        deficit = [0.0]

        def pop_fillers(fillers):
            # best-fit: emit the first queued unit that fits the PE deficit.
            # V units insert a 612ns copy into the ACT stream, which extends
            # the attention window by that much -> credit it back.
            while fillers:
                pick = None
                for i, (cost, _, _) in enumerate(fillers):
                    if cost <= deficit[0]:
                        pick = i
                        break
                if pick is None:
                    return
                cost, _, thunk = fillers.pop(pick)
                thunk()
                deficit[0] -= cost
                if cost == 854.0 and thunk.__qualname__.find("v_unit") >= 0:
                    deficit[0] += VCOPY_ACT

        def drain_needed(fillers, level):
            # force-emit every unit that must land before attention chunk
            # `level` (its qrot/krot/v_all inputs are read there)
            rest = []
            for cost, need, thunk in fillers:
                if need <= level:
                    thunk()
                else:
                    rest.append((cost, need, thunk))
            fillers[:] = rest

        def attention_chunk(j, fillers, tail=(), norm_out=None,
                            inline_v=None):
            # fillers: (pe_ns, thunk) work emitted into ACT-gated iterations.
            # tail: units emitted just before the last pair's final PV so
            # they fill the post-last-exp norm window (emitting them after
            # the chunk would queue them behind the norm's sem chain).
            # norm_out: if given, pair-norm chains are appended as thunks
            # instead of emitted (caller runs them after boundary ropes).
            # inline_v: {t: v_unit thunk} emitted just before pair-0's PV(t)
            # (chunk 0 builds V inside the attention stream).
            nt = 4 * (j + 1)
            qs0 = SQ * j
            for p in range(2):
                if p == 1:
                    drain_needed(fillers, j + 0.5)
                pa = ps_pa.tile([128, 1024], F32, name="pa", tag="pa")
                pend = {}

                def qk(t):
                    r = t - 4 * j
                    c0 = 0 if r < 0 else 128 * r
                    psc = ps_big.tile([128, 1024], F32, name="psc", tag="big")
                    for hh in range(2):
                        nc.tensor.matmul(
                            psc[:, 512 * hh + c0:512 * hh + 512],
                            krot[p][64 * hh:64 * (hh + 1),
                                    128 * t:128 * (t + 1)],
                            qrot[p][64 * hh:64 * (hh + 1), qs0 + c0:qs0 + SQ],
                            start=True, stop=True)
                    pend[t] = (psc, c0, r)

                for t in range(min(LOOK, nt)):
                    qk(t)
                for t in range(nt):
                    psc, c0, r = pend.pop(t)
                    if r >= 0:
                        pv = psc[:].rearrange("q (h n) -> q h n",
                                              h=2)[:, :, c0:c0 + 128]
                        trib = tri[:].unsqueeze(1).broadcast_to((128, 2, 128))
                        nc.vector.tensor_tensor(pv, pv, trib, ADD)
                    e = e_pool.tile([128, 1024], BF16, name="e", tag="e")
                    ev = e[:].rearrange("q (h n) -> q h n", h=2)[:, :, c0:SQ]
                    pvv = psc[:].rearrange("q (h n) -> q h n",
                                           h=2)[:, :, c0:SQ]
                    nc.scalar.activation(ev, pvv, Exp, scale=0.125)
                    n_live = 512 - c0
                    pe_ns = 2 * n_live * 0.4167
                    if t + LOOK < nt:
                        rl = t + LOOK - 4 * j
                        pe_ns += 2 * (512 - (0 if rl < 0 else 128 * rl)) \
                            * 0.4167
                    deficit[0] += (2 * n_live * 0.833 + 215) - pe_ns
                    if r >= 0:
                        deficit[0] += 250.0
                    # fillers go BEFORE qk(t+LOOK): the PE queue is in-order
                    # and qk blocks on the psc pool slot (freed by exp(t)),
                    # so anything emitted after it would stall behind it.
                    pop_fillers(fillers)
                    if t + LOOK < nt:
                        qk(t + LOOK)
                    if p == 1 and t == nt - 1:
                        for tu in tail:
                            tu()
                    if inline_v is not None and p == 0 and t in inline_v:
                        inline_v.pop(t)()
                    for hh in range(2):
                        h = 2 * p + hh
                        nc.tensor.matmul(
                            pa[0:VW, 512 * hh + c0:512 * hh + 512],
                            v_all[:, (t * HPC + h) * VW:
                                  (t * HPC + h + 1) * VW],
                            e[:, 512 * hh + c0:512 * hh + 512],
                            start=(t == 0), stop=(t == nt - 1))
                # fast pa release: copy psum -> sbuf, normalize off-psum.
                # Final pair skips the copy: nothing queues behind it.
                last = (j == 3 and p == 1)
                if last:
                    pac = pa
                else:
                    # split evacuation across DVE+ACT so pa frees in ~660ns
                    # (a single DVE copy holds it for ~1.2us)
                    pac = pac_pool.tile([128, 1024], BF16, name="pac",
                                        tag="pac")
                    nc.vector.tensor_copy(pac[:, 0:512], pa[:, 0:512])
                    nc.scalar.copy(pac[:, 512:1024], pa[:, 512:1024])

                def norm_emit(p=p, pac=pac, last=last):
                    # bf16 recip/broadcast: the anorm muls then run with
                    # all-2-byte SBUF operands -> DVE 2x mode (327 vs 594).
                    # 1/Z at bf16 adds ~0.4% noise; tolerance is 2e-2.
                    with nc.allow_low_precision(
                            reason="bf16 softmax denominators, tol 2e-2"):
                        _norm_emit(p, pac, last)

                def _norm_emit(p, pac, last):
                    rcp = div_pool.tile([1, 1024], BF16, name="rcp",
                                        tag="rcp")
                    rb = div_pool.tile([64, 1024], BF16, name="rb", tag="rb")

                    def tickle(src):
                        # tiny matmul reading a norm-chain intermediate:
                        # keeps the PE ramp-tracker alive through the tail
                        # norm window (unlike real fillers, it cannot be
                        # hoisted earlier by the scheduler)
                        tk = ps_big.tile([128, 1024], F32, name="tk",
                                         tag="big")
                        nc.tensor.matmul(tk[0:8, 0:8], src, src, start=True,
                                         stop=True)
                    if last:
                        # tail-critical: split recip so bcast/mul pipeline
                        for hh in range(2):
                            nc.vector.reciprocal(
                                rcp[:, 512 * hh:512 * (hh + 1)],
                                pac[HD:HD + 1, 512 * hh:512 * (hh + 1)])
                    else:
                        nc.vector.reciprocal(rcp[:], pac[HD:HD + 1, :])
                    # split broadcast so mul hh=0 overlaps broadcast hh=1
                    for hh in range(2):
                        nc.gpsimd.partition_broadcast(
                            rb[:, 512 * hh:512 * (hh + 1)],
                            rcp[:, 512 * hh:512 * (hh + 1)])
                        if last:
                            tickle(rb[0:8, 512 * hh:512 * hh + 8])
                        nc.vector.tensor_tensor(
                            anorm[p][j][64 * hh:64 * (hh + 1), 0:SQ],
                            pac[0:HD, 512 * hh:512 * (hh + 1)],
                            rb[:, 512 * hh:512 * (hh + 1)], MUL)
                        if last:
                            tickle(anorm[p][j][0:8, 8 * hh:8 * hh + 8])
                if norm_out is None or j == 3:
                    norm_emit()
                else:
                    # defer the norm chain: its DVE/Pool ops otherwise queue
                    # ahead of the next chunk's boundary-critical rope work
                    norm_out.append(norm_emit)
                # norm window + the pac ACT-half copy extend the window
                deficit[0] += NORM_CREDIT
                pop_fillers(fillers)

        # chunk 0 emitted directly; everything else threads through the
        # filler queue so PE stays dense during the ACT-gated attention.
        # Order: pair-0 Q/K first, then V tiles + pair-1 Q/K, so pair-0's
        # rope chain (DVE+gpsimd) overlaps the V/pair-1 matmuls.
        u0 = proj_units(0)
        u0[0]()
        u0[1]()
        for u in v_units(0):
            u()
        u0[2]()
        u0[3]()
        PC, VC, OC2 = 1707.0, 854.0, 854.0
        fillq = []
        fillq += [(PC, 1, u) for u in proj_units(1)]
        fillq += [(VC, 1, u) for u in v_units(1)]
        fillq += [(PC, 2, u) for u in proj_units(2)]
        fillq += [(VC, 2, u) for u in v_units(2)]
        norms = []
        attention_chunk(0, fillq, norm_out=norms)
        drain_needed(fillq, 1)
        for nrm in norms:
            nrm()
        norms = []
        fillq += [(PC, 3, u) for u in proj_units(3)]
        fillq += [(VC, 3, u) for u in v_units(3)]
        fillq += [(OC2, 9, po_unit(0, u)) for u in range(4)]
        attention_chunk(1, fillq, norm_out=norms)
        drain_needed(fillq, 2)
        for nrm in norms:
            nrm()
        norms = []
        fillq += [(OC2, 9, po_unit(1, u)) for u in range(4)]
        attention_chunk(2, fillq, norm_out=norms)
        drain_needed(fillq, 3)
        for nrm in norms:
            nrm()
        # hold back most of po(2): it fills the post-last-exp norm window
        fillq += [(OC2, 9, po_unit(2, 0))]
        attention_chunk(3, fillq,
                        tail=[po_unit(2, u) for u in range(1, 4)])
        for _, _, u in fillq:
            u()
        for u in range(4):
            po_unit(3, u)()


def build_nc():
    nc = bacc.Bacc("TRN2", target_bir_lowering=False, debug=False,
                   num_devices=NCORES)
    xP = nc.dram_tensor("xP", [128, KD * S], BF16, kind="ExternalInput").ap()
    wqP = nc.dram_tensor("wqP", [128, KD * CL], BF16,
                         kind="ExternalInput").ap()
    wkP = nc.dram_tensor("wkP", [128, KD * CL], BF16,
                         kind="ExternalInput").ap()
    wvP = nc.dram_tensor("wvP", [128, KD * CL], BF16,
                         kind="ExternalInput").ap()
    woP = nc.dram_tensor("woP", [128, 2 * D], BF16, kind="ExternalInput").ap()
    cosP = nc.dram_tensor("cosP", [128, S], BF16, kind="ExternalInput").ap()
    sinP = nc.dram_tensor("sinP", [128, S], BF16, kind="ExternalInput").ap()
    triP = nc.dram_tensor("triP", [128, 128], F32, kind="ExternalInput").ap()
    outP = nc.dram_tensor("outP", [128, KD * S], BF16,
                          kind="ExternalOutput").ap()
    with tile.TileContext(nc) as tc:
        _build_body(nc, tc, xP, wqP, wkP, wvP, woP, cosP, sinP, triP, outP)
    nc.compile()
    return nc


def host_constants():
    """RoPE cos/sin tiles (T layout, sign folded into sin) + [128,128] tri."""
    freqs = 1.0 / (THETA ** (np.arange(0, HD, 2, dtype=np.float32)
                             / np.float32(HD)))
    pos = np.arange(S, dtype=np.float32)
    ang = pos[:, None] * freqs[None, :]          # [S, 32]
    cos = np.cos(ang).astype(np.float32)
    sin = np.sin(ang).astype(np.float32)
    rows_i = (np.arange(128) % HD) // 2
    cosT = np.ascontiguousarray(cos[:, rows_i].T)          # [128, S]
    sgn = np.where(np.arange(128) % 2 == 0, -1.0, 1.0).astype(np.float32)
    sinT = np.ascontiguousarray(sin[:, rows_i].T * sgn[:, None])
    p = np.arange(128)[:, None]
    tri = np.where(np.arange(128)[None, :] >= p, 0.0, -1e9).astype(np.float32)
    return cosT, sinT, tri


def _pack(mat, kchunks):
    """[kchunks*128, W] -> [128, kchunks*W] partition-major image."""
    kw = mat.shape[1]
    return np.ascontiguousarray(
        mat.reshape(kchunks, 128, kw).transpose(1, 0, 2).reshape(
            128, kchunks * kw))


def make_in_maps(x, wq, wk, wv, wo):
    import ml_dtypes
    bf = ml_dtypes.bfloat16
    cosT, sinT, tri = host_constants()
    in_maps = []
    for c in range(NCORES):
        b, g = divmod(c, 4)
        cs = slice(CL * g, CL * (g + 1))
        xPm = _pack(np.ascontiguousarray(x[b].T), KD).astype(bf)
        wqPm = _pack(np.ascontiguousarray(wq[cs, :].T), KD).astype(bf)
        wkPm = _pack(np.ascontiguousarray(wk[cs, :].T), KD).astype(bf)
        wvPm = _pack(np.ascontiguousarray(wv[cs, :].T), KD).astype(bf)
        woPm = _pack(np.ascontiguousarray(wo[:, cs].T), 2).astype(bf)
        in_maps.append({
            "xP": xPm, "wqP": wqPm, "wkP": wkPm, "wvP": wvPm, "woP": woPm,
            "cosP": cosT.astype(bf), "sinP": sinT.astype(bf), "triP": tri,
        })
    return in_maps


_CACHE = {}
TRACE = False


def kernel(x, q_proj_weight, k_proj_weight, v_proj_weight, o_proj_weight):
    from concourse.bass_utils import run_bass_kernel_spmd
    x = np.asarray(x, dtype=np.float32)
    in_maps = make_in_maps(x, np.asarray(q_proj_weight, dtype=np.float32),
                           np.asarray(k_proj_weight, dtype=np.float32),
                           np.asarray(v_proj_weight, dtype=np.float32),
                           np.asarray(o_proj_weight, dtype=np.float32))
    if "nc" not in _CACHE:
        _CACHE["nc"] = build_nc()
    res = run_bass_kernel_spmd(_CACHE["nc"], in_maps,
                               core_ids=list(range(NCORES)), trace=TRACE)
    _CACHE["last_results"] = res
    out = np.zeros((B, S, D), dtype=np.float32)
    for c in range(NCORES):
        o = np.asarray(res.results[c]["outP"]).astype(np.float32)
        # o[p, mt*S + s] = partial out[b][s, 128*mt + p]
        o = o.reshape(128, KD, S).transpose(2, 1, 0).reshape(S, D)
        out[c // 4] += o
    return out

